# revision 13
# baseline (speedup 1.0000x reference)
"""Trainium2 Bass kernel for nn_AttentionConvInput.

Math (per batch b):
    A[i,j]  = 1 / (1 + ||x0[b,0,i] - x1[b,0,j]||)          [1024 x 1024]
    a0      = A @ W0,  a1 = A.T @ W1                        [1024 x 128]
    f0      = concat([x0, a0], ch), f1 = concat([x1, a1], ch)

Strategy:
  - Data-parallel over batch: 4 batches per NeuronCore x 8 cores.
  - Host pre-transposes x0/x1 to [D, L] bf16 (x1 scaled by -2) and
    precomputes squared-norm rows (fp16), so the device computes
        d2 = (sq_a[i] + sq_b[j]) + x0T.T @ (-2*x1T)
    via PSUM accumulation of a K=2 fp16 rank-2 matmul + K=128 bf16 matmul.
    (fp32r is deliberately avoided: fp32r weight loads corrupt concurrent
    DMA xbar transposes.)
  - ACT: t = sqrt(d2) (PSUM->SBUF). DVE: t += 1; A = recip_approx(t) -> bf16.
  - A.T via 8 big DMA xbar transposes per batch ([128,1024] -> 3D dest).
  - a1T[n,j] = sum_i W1[i,n]*A[i,j]  (lhsT = W1 blocks, rhs = A natural)
    a0T[n,i] = sum_j W0[j,n]*AT[j,i] (lhsT = W0 blocks, rhs = A transposed)
  - ACT copies PSUM results to SBUF; DMA out; host reassembles + concats.
"""

import numpy as np
import ml_dtypes

B, C, L, D = 32, 1, 1024, 128
N_CORES = 8
BPC = B // N_CORES  # batches per core

_CACHE = {}


def _make_recip1p():
    """Fused custom DVE op: out ~= 1/(imm2 + x) in ONE pass.
    Bit-trick seed (x*bitcast(~x) in [-4.5,-4]) + one Newton step.
    Max rel err ~1.8e-3. 6 of 8 DVE slices."""
    if "recip1p" in _CACHE:
        return _CACHE["recip1p"]
    import re
    import numpy as np
    from concourse import dve_ops
    from concourse.dve_spec import AluOp, Bin, C0, C1, C2, Spec, Src0

    def _ref(in0, in1, c0, c1, c2):
        u = in0.astype(np.float32) + np.float32(c2)
        w = (~u.view(np.int32)).view(np.float32)
        y0 = (w * np.float32(c0)).astype(np.float32)
        return (y0 * (np.float32(c1) - u * y0)).astype(np.float32)

    _u = Src0 + C2
    _w = Bin(AluOp.BITWISE_NOT, _u, _u)
    _y0 = _w * C0
    spec = Spec(body=_y0 * (C1 - _u * _y0), reference=_ref)

    shas = {}
    for ver in ("v3", "v4"):
        probe = dve_ops.DveOp("RECIP1P_ANT", spec, subdim=False, uops_sha={})
        # temporarily register so compile can resolve the opcode row
        row = max(dve_ops._SUB_OPCODE_FOR_NAME.values()) + 1
        dve_ops._SUB_OPCODE_FOR_NAME.setdefault("RECIP1P_ANT", row)
        try:
            probe.compile(ver)
        except ValueError as e:
            m = re.search(r"\(%s: ([0-9a-f]+)" % ver, str(e))
            shas[ver] = m.group(1)
    op = dve_ops.DveOp("RECIP1P_ANT", spec, subdim=False, uops_sha=shas)
    if all(o.name != "RECIP1P_ANT" for o in dve_ops.OPS):
        dve_ops.OPS.append(op)
    dve_ops.CUSTOM_DVE_SPECS["RECIP1P_ANT"] = spec
    _CACHE["recip1p"] = op
    return op


RECIP1P_C0 = -0.23550000
RECIP1P_C1 = 2.00170000


def _build(loop_n=None):
    from contextlib import ExitStack

    import concourse.bacc as bacc
    import concourse.mybir as mybir
    import concourse.tile as tile

    dt = mybir.dt
    AF = mybir.ActivationFunctionType
    recip1p = _make_recip1p()

    nc = bacc.Bacc(
        "TRN2",
        target_bir_lowering=False,
        debug=False,
        enable_asserts=False,
    )

    # host packs x0T and -2*x1T side by side: [BPC, 128, 2048]
    xx = nc.dram_tensor("xx", [BPC, 128, 2048], dt.bfloat16, kind="ExternalInput").ap()
    # aug rows: [sq_a; ones; ones; sq_b] : [BPC, 4, 1024]
    aug = nc.dram_tensor("aug", [BPC, 4, 1024], dt.float16, kind="ExternalInput").ap()
    w0 = nc.dram_tensor("w0", [128, 8, 128], dt.bfloat16, kind="ExternalInput").ap()
    w1 = nc.dram_tensor("w1", [128, 8, 128], dt.bfloat16, kind="ExternalInput").ap()
    a0o = nc.dram_tensor("a0o", [BPC, 128, 1024], dt.float32, kind="ExternalOutput").ap()
    a1o = nc.dram_tensor("a1o", [BPC, 128, 1024], dt.float32, kind="ExternalOutput").ap()

    with ExitStack() as ctx:
        tc = ctx.enter_context(tile.TileContext(nc))

        w_pool = ctx.enter_context(tc.tile_pool(name="w", bufs=1))
        x_pool = ctx.enter_context(tc.tile_pool(name="x", bufs=2))
        aug_pool = ctx.enter_context(tc.tile_pool(name="augp", bufs=2))
        t_pool = ctx.enter_context(tc.tile_pool(name="t", bufs=6))
        a_pool = ctx.enter_context(tc.tile_pool(name="amat", bufs=2))
        at_pool = ctx.enter_context(tc.tile_pool(name="atmat", bufs=2))
        o_pool = ctx.enter_context(tc.tile_pool(name="o", bufs=4))
        ps_d2 = ctx.enter_context(tc.tile_pool(name="psd2", bufs=2, space="PSUM"))
        ps_o = ctx.enter_context(tc.tile_pool(name="pso", bufs=2, space="PSUM"))

        w0_sb = w_pool.tile([128, 8, 128], dt.bfloat16, tag="w0")
        w1_sb = w_pool.tile([128, 8, 128], dt.bfloat16, tag="w1")
        nc.sync.dma_start(w0_sb, w0)
        nc.sync.dma_start(w1_sb, w1)

        def emit_a0(b, at_raw):
            for isd in range(2):
                isl = slice(isd * 512, (isd + 1) * 512)
                pa0 = ps_o.tile([128, 512], dt.float32, tag="pa0", name=f"pa0_{b}_{isd}")
                for jb in range(8):
                    g0 = isd * 32 + jb
                    nc.tensor.matmul(pa0, w0_sb[:, jb, :], at_raw[:, g0:g0 + 25:8, :],
                                     start=(jb == 0), stop=(jb == 7))
                o0 = o_pool.tile([128, 512], dt.float32, tag="o0", name=f"o0_{b}_{isd}")
                nc.scalar.copy(o0, pa0)
                nc.sync.dma_start(a0o[b][:, isl], o0)

        def body():
            pending = []
            for b in range(BPC):
                xx_sb = x_pool.tile([128, 2048], dt.bfloat16, tag="xx", name=f"xx_{b}")
                nc.sync.dma_start(xx_sb, xx[b])
                aa = aug_pool.tile([2, 1024], dt.float16, tag="aa", name=f"aa_{b}")
                ab = aug_pool.tile([2, 1024], dt.float16, tag="ab", name=f"ab_{b}")
                nc.sync.dma_start(aa, aug[b, 0:2])
                nc.sync.dma_start(ab, aug[b, 2:4])

                a_big = a_pool.tile([128, 8, 1024], dt.bfloat16, tag="A", name=f"A{b}")
                at_raw = at_pool.tile([128, 64, 128], dt.bfloat16, tag="AT", name=f"AT{b}")

                for js in range(2):
                    jsl = slice(js * 512, (js + 1) * 512)
                    for ih in range(4):
                        # fat psum tiles: two i-blocks per ACT op
                        ps = ps_d2.tile([128, 2, 512], dt.float32, tag="d2", name=f"d2_{b}_{js}_{ih}")
                        tt = t_pool.tile([128, 2, 512], dt.float32, tag="t", name=f"t_{b}_{js}_{ih}")
                        for k in range(2):
                            ib = ih * 2 + k
                            ibl = slice(ib * 128, (ib + 1) * 128)
                            nc.tensor.matmul(ps[:, k], aa[:, ibl], ab[:, jsl], start=True, stop=False)
                            nc.tensor.matmul(ps[:, k], xx_sb[:, ibl],
                                             xx_sb[:, 1024 + jsl.start:1024 + jsl.stop],
                                             start=False, stop=True)
                        nc.scalar.activation(tt, ps, AF.Sqrt)
                        for k in range(2):
                            ib = ih * 2 + k
                            nc.vector._custom_dve(
                                recip1p, out=a_big[:, ib, jsl], in0=tt[:, k],
                                s0=RECIP1P_C0, s1=RECIP1P_C1, imm2=1.0,
                            )
                    pa1 = ps_o.tile([128, 512], dt.float32, tag="pa1", name=f"pa1_{b}_{js}")
                    for ib in range(8):
                        nc.tensor.matmul(pa1, w1_sb[:, ib, :], a_big[:, ib, jsl],
                                         start=(ib == 0), stop=(ib == 7))
                    o1 = o_pool.tile([128, 512], dt.float32, tag="o1", name=f"o1_{b}_{js}")
                    nc.scalar.copy(o1, pa1)
                    nc.sync.dma_start(a1o[b][:, jsl], o1)

                # A^T: ONE xbar transpose for the whole batch
                nc.sync.dma_start_transpose(at_raw, a_big)
                pending.append((b, at_raw))
                if len(pending) > 1:
                    emit_a0(*pending.pop(0))
            for p in pending:
                emit_a0(*p)

        if loop_n is None:
            body()
        else:
            with tc.For_i(0, loop_n, 1):
                body()

    nc.compile()
    return nc


def _get_nc():
    if "nc" not in _CACHE:
        _CACHE["nc"] = _build()
    return _CACHE["nc"]


def make_in_maps(x0, x1, W0, W1):
    bf16 = ml_dtypes.bfloat16
    a = x0[:, 0]                                    # [B, L, D]
    bm = x1[:, 0]
    xx_full = np.empty((B, 128, 2048), dtype=bf16)
    xx_full[:, :, :1024] = a.transpose(0, 2, 1).astype(bf16)
    xx_full[:, :, 1024:] = (-2.0 * bm).transpose(0, 2, 1).astype(bf16)
    sqa = np.sum(a.astype(np.float64) ** 2, axis=-1).astype(np.float32)
    sqb = np.sum(bm.astype(np.float64) ** 2, axis=-1).astype(np.float32)
    ones = np.ones((B, L), np.float32)
    aug_full = np.stack([sqa, ones, ones, sqb], axis=1).astype(np.float16)
    w0_blocks = np.ascontiguousarray(W0.reshape(8, 128, 128).transpose(1, 0, 2)).astype(bf16)
    w1_blocks = np.ascontiguousarray(W1.reshape(8, 128, 128).transpose(1, 0, 2)).astype(bf16)

    in_maps = []
    for c in range(N_CORES):
        s = slice(c * BPC, (c + 1) * BPC)
        in_maps.append({
            "xx": np.ascontiguousarray(xx_full[s]),
            "aug": np.ascontiguousarray(aug_full[s]),
            "w0": w0_blocks,
            "w1": w1_blocks,
        })
    return in_maps


def kernel(x0, x1, W0, W1):
    from concourse.bass_utils import run_bass_kernel_spmd

    x0 = np.asarray(x0, dtype=np.float32)
    x1 = np.asarray(x1, dtype=np.float32)
    W0 = np.asarray(W0, dtype=np.float32)
    W1 = np.asarray(W1, dtype=np.float32)

    in_maps = make_in_maps(x0, x1, W0, W1)
    nc = _get_nc()
    _CACHE["in_maps"] = in_maps
    res = run_bass_kernel_spmd(nc, in_maps, core_ids=list(range(N_CORES)))

    a0T = np.concatenate([res.results[c]["a0o"] for c in range(N_CORES)], axis=0)
    a1T = np.concatenate([res.results[c]["a1o"] for c in range(N_CORES)], axis=0)

    a0 = a0T.transpose(0, 2, 1)[:, None]            # [B, 1, L, D]
    a1 = a1T.transpose(0, 2, 1)[:, None]
    f0 = np.concatenate([x0, a0], axis=1)
    f1 = np.concatenate([x1, a1], axis=1)
    return (f0, f1)
